# revision 1
# baseline (speedup 1.0000x reference)
"""GCN forward on 8 TRN2 NeuronCores — host prep + Bass/Tile builder + runner.

Model (reference.py): 2-layer GCN, N=100000 nodes, E=1600000 edges,
IN=HID=128, OUT=64, symmetric normalization with self-loops.

Math:
  deg[t] = in_degree(t)+1 ; dinv = deg^-1/2
  table1[s] = dinv[s]*(x@W1)[s] ; y1[t] = relu(dinv[t]*sum_e table1[row_e] + b1)
  table2[s] = dinv[s]*y1[s]     ; out[t] = (dinv[t]*sum_e table2[row_e])@W2 + b2
"""

import sys

sys.path.insert(0, "/opt/trn_rl_repo")
import numpy as np
import ml_dtypes

import concourse.bass as bass
import concourse.mybir as mybir
import concourse.tile as tile
from concourse import bacc
from concourse.bass_utils import run_bass_kernel_spmd

F32 = mybir.dt.float32
BF16 = mybir.dt.bfloat16
I16 = mybir.dt.int16
I32 = mybir.dt.int32
AF = mybir.ActivationFunctionType
ALU = mybir.AluOpType

P = 128
N, E = 100000, 1600000
IN, HID, OUT = 128, 128, 64
NCORES = 8
TPC = 12500
PPC = 12544
NPAD = PPC * NCORES
NCHUNK = 4
CHUNK = NPAD // NCHUNK
NG = PPC // P
SG = 12
MAX_CALL_BLOCKS = 48


def host_prep(edge_index):
    row = np.asarray(edge_index[0], dtype=np.int64)
    col = np.asarray(edge_index[1], dtype=np.int64)
    deg = np.bincount(col, minlength=N).astype(np.int64) + 1

    n_all = np.arange(N, dtype=np.int64)
    gids = (n_all // TPC) * PPC + (n_all % TPC)
    grow = np.concatenate([gids[row], gids])
    gcol = np.concatenate([gids[col], gids])

    owner = gcol // PPC
    g_all = (gcol - owner * PPC) >> 7
    loc_all = (gcol - owner * PPC) & 127
    ch_all = grow // CHUNK
    lidx_all = (grow % CHUNK).astype(np.int16)

    NSEG = NG * NCHUNK
    counts = np.zeros((NCORES, NSEG), np.int64)
    per_core = []
    for c in range(NCORES):
        sel = owner == c
        g = g_all[sel]
        ch = ch_all[sel]
        order = np.lexsort((ch, g))
        seg = (g * NCHUNK + ch)[order]
        counts[c] = np.bincount(seg, minlength=NSEG)
        per_core.append((seg, lidx_all[sel][order], loc_all[sel][order]))

    nb_seg = -(-counts.max(axis=0) // P)
    nb_seg = np.maximum(nb_seg, (np.arange(NSEG) % NCHUNK == 0).astype(np.int64))
    NB = nb_seg.reshape(NG, NCHUNK)

    sgs = []
    g0 = 0
    while g0 < NG:
        sgs.append((g0, min(SG, NG - g0)))
        g0 += SG
    seg_base = np.zeros(NSEG, np.int64)
    calls = []
    tot = 0
    for sgi, (gs, ng) in enumerate(sgs):
        for ch in range(NCHUNK):
            call_start = tot
            for gl in range(ng):
                g = gs + gl
                seg_base[g * NCHUNK + ch] = tot
                tot += NB[g, ch]
            nb_call = tot - call_start
            off = call_start
            while nb_call > 0:
                take = min(nb_call, MAX_CALL_BLOCKS)
                calls.append((sgi, ch, off, take))
                off += take
                nb_call -= take
    TOTB = tot

    idx_list, colv_list = [], []
    for c in range(NCORES):
        seg, lidx, loc = per_core[c]
        seg_start = np.zeros(NSEG, np.int64)
        seg_start[1:] = np.cumsum(counts[c])[:-1]
        rank = np.arange(seg.shape[0]) - seg_start[seg]
        dest = seg_base[seg] * P + rank
        idx_arr = np.zeros(TOTB * P, np.int16)
        colv_arr = np.full(TOTB * P, -1.0, np.float32)
        idx_arr[dest] = lidx
        colv_arr[dest] = loc.astype(np.float32)
        idx_list.append(np.tile(idx_arr.reshape(-1, 16).T, (8, 1)).copy())
        colv_list.append(
            np.ascontiguousarray(colv_arr.reshape(TOTB, P).T.astype(ml_dtypes.bfloat16))
        )

    degp = np.ones(NPAD, np.int32)
    degp[gids] = deg.astype(np.int32)
    dego_list = [
        np.ascontiguousarray(degp[c * PPC : (c + 1) * PPC].reshape(NG, P).T)
        for c in range(NCORES)
    ]
    # blockmap: blk -> (gl_in_sg, is_first, is_last)
    last_ch = [int(np.nonzero(NB[g])[0][-1]) for g in range(NG)]
    blockmap = {}
    for sgi, (gs, ng) in enumerate(sgs):
        for ch in range(NCHUNK):
            for gl in range(ng):
                g = gs + gl
                base = int(seg_base[g * NCHUNK + ch])
                nbg = int(NB[g, ch])
                for k in range(nbg):
                    blockmap[base + k] = (
                        gl,
                        ch == 0 and k == 0,
                        ch == last_ch[g] and k == nbg - 1,
                    )
    sched = {
        "NB": NB,
        "sgs": sgs,
        "calls": calls,
        "TOTB": TOTB,
        "seg_base": seg_base,
        "blockmap": blockmap,
    }
    return sched, idx_list, colv_list, dego_list, gids


def build_kernel(sched, stage=9):
    NB, sgs, calls, TOTB = sched["NB"], sched["sgs"], sched["calls"], sched["TOTB"]
    blockmap = sched["blockmap"]

    nc = bacc.Bacc("TRN2", target_bir_lowering=False, num_devices=NCORES)
    xT = nc.dram_tensor("xT", [P, PPC], F32, kind="ExternalInput")
    dego = nc.dram_tensor("dego", [P, NG], I32, kind="ExternalInput")
    W1 = nc.dram_tensor("W1", [IN, HID], F32, kind="ExternalInput")
    W2 = nc.dram_tensor("W2", [HID, OUT], F32, kind="ExternalInput")
    b1r = nc.dram_tensor("b1r", [P, HID], F32, kind="ExternalInput")
    b2r = nc.dram_tensor("b2r", [P, OUT], F32, kind="ExternalInput")
    iota = nc.dram_tensor("iota", [P, P], BF16, kind="ExternalInput")
    idx = nc.dram_tensor("idx", [P, TOTB * 8], I16, kind="ExternalInput")
    colv = nc.dram_tensor("colv", [P, TOTB], BF16, kind="ExternalInput")
    y = nc.dram_tensor("y", [P, NG, OUT], F32, kind="ExternalOutput")

    with tile.TileContext(nc) as tc:
        with (
            tc.tile_pool(name="const", bufs=1) as cpool,
            tc.tile_pool(name="sb", bufs=2) as sb,
            tc.tile_pool(name="dram", bufs=1, space="DRAM") as dpool,
            tc.tile_pool(name="psX", bufs=2, space="PSUM") as psX,
            tc.tile_pool(name="psAgg", bufs=2, space="PSUM") as psAgg,
        ):
            # ---- constants ----
            W1b = cpool.tile([IN, HID], BF16)
            nc.gpsimd.dma_start(out=W1b[:], in_=W1[:])
            W2b = cpool.tile([HID, OUT], BF16)
            nc.gpsimd.dma_start(out=W2b[:], in_=W2[:])
            b1t = cpool.tile([P, HID], F32)
            nc.sync.dma_start(out=b1t[:], in_=b1r[:])
            b2t = cpool.tile([P, OUT], F32)
            nc.sync.dma_start(out=b2t[:], in_=b2r[:])
            iota_t = cpool.tile([P, P], BF16)
            nc.sync.dma_start(out=iota_t[:], in_=iota[:])
            idx_t = cpool.tile([P, TOTB * 8], I16)
            nc.sync.dma_start(out=idx_t[:], in_=idx[:])
            colv_t = cpool.tile([P, TOTB], BF16)
            nc.sync.dma_start(out=colv_t[:], in_=colv[:])

            dego_i = cpool.tile([P, NG], I32)
            nc.sync.dma_start(out=dego_i[:], in_=dego[:])
            dego_f = cpool.tile([P, NG], F32)
            nc.vector.tensor_copy(out=dego_f[:], in_=dego_i[:])
            dsq = cpool.tile([P, NG], F32)
            nc.scalar.activation(dsq[:], dego_f[:], AF.Sqrt)
            dinv = cpool.tile([P, NG], F32)
            nc.vector.reciprocal(out=dinv[:], in_=dsq[:])

            # ---- phase A ----
            t1in = dpool.tile([PPC, HID], BF16)
            table1 = dpool.tile([NPAD, HID], BF16)
            if stage >= 1:
                XC = 14
                for c0 in range(0, NG, XC):
                    xbf = sb.tile([P, XC * P], BF16, tag="xbf", name="xbf")
                    nc.gpsimd.dma_start(out=xbf[:], in_=xT[:, c0 * P : (c0 + XC) * P])
                    t1s = sb.tile([P, XC, HID], BF16, tag="t1s", name="t1s")
                    for b in range(XC):
                        ps = psX.tile([P, HID], F32, tag="px", name="ps")
                        nc.tensor.matmul(
                            out=ps[:],
                            lhsT=xbf[:, b * P : (b + 1) * P],
                            rhs=W1b[:],
                            start=True,
                            stop=True,
                        )
                        nc.scalar.activation(
                            t1s[:, b, :], ps[:], AF.Copy,
                            scale=dinv[:, c0 + b : c0 + b + 1],
                        )
                    nc.sync.dma_start(
                        out=t1in.rearrange("(n p) f -> p n f", p=P)[:, c0 : c0 + XC, :],
                        in_=t1s[:],
                    )
            if stage >= 2:
                nc.gpsimd.collective_compute(
                    "AllGather", ALU.bypass, ins=[t1in[:]], outs=[table1[:]],
                    replica_groups=[list(range(NCORES))],
                )

            agin = dpool.tile([PPC, HID], BF16)
            table2 = dpool.tile([NPAD, HID], BF16)

            def agg_layer(table, orientation, epilogue, nsg):
                for sgi, (gs, ng) in enumerate(sgs[:nsg]):
                    nbank = -(-ng // 4)
                    banks = [
                        psAgg.tile([P, 512], F32, name=f"bank{i}", tag=f"aggbank{i}", bufs=2)
                        for i in range(nbank)
                    ]
                    for bk in banks:
                        nc.vector.memset(bk[:], 0.0)
                    for ch in range(NCHUNK):
                        for (csgi, cch, boff, nb) in calls:
                            if csgi != sgi or cch != ch:
                                continue
                            msgs = sb.tile([P, nb, HID], BF16, tag="msgs", bufs=3, name="msgs")
                            nc.gpsimd.dma_gather(
                                msgs[:],
                                table[ch * CHUNK : (ch + 1) * CHUNK, :],
                                idx_t[:, boff * 8 : (boff + nb) * 8],
                                nb * P,
                                nb * P,
                                HID,
                                single_packet=False,
                            )
                            S = sb.tile([P, nb, P], BF16, tag="S", bufs=3, name="S")
                            nc.vector.tensor_tensor(
                                out=S[:],
                                in0=colv_t[:, boff : boff + nb, None].to_broadcast([P, nb, P]),
                                in1=iota_t[:, None, :].to_broadcast([P, nb, P]),
                                op=ALU.is_equal,
                            )
                            for k in range(nb):
                                blk = boff + k
                                gl, is_first, is_last = blockmap[blk]
                                region = banks[gl // 4][:, (gl % 4) * P : (gl % 4 + 1) * P]
                                if orientation == 1:
                                    nc.tensor.matmul(
                                        out=region, lhsT=S[:, k, :], rhs=msgs[:, k, :],
                                        start=False, stop=is_last, skip_group_check=True,
                                    )
                                else:
                                    nc.tensor.matmul(
                                        out=region, lhsT=msgs[:, k, :], rhs=S[:, k, :],
                                        start=False, stop=is_last, skip_group_check=True,
                                    )
                    for gl in range(ng):
                        epilogue(
                            sgi, gs + gl, gl, ng,
                            banks[gl // 4][:, (gl % 4) * P : (gl % 4 + 1) * P],
                        )

            # ---- L1 ----
            y1sg = {}

            def epi1(sgi, g, gl, ng, region):
                if gl == 0:
                    y1sg[sgi] = sb.tile([P, ng, HID], BF16, name="y1s", tag="y1s", bufs=2)
                tmp = sb.tile([P, HID], F32, tag="epi1a", bufs=2, name="tmp")
                nc.scalar.activation(tmp[:], region, AF.Copy, scale=dinv[:, g : g + 1])
                tmp2 = sb.tile([P, HID], F32, tag="epi1b", bufs=2, name="tmp2")
                nc.vector.tensor_tensor(out=tmp2[:], in0=tmp[:], in1=b1t[:], op=ALU.add)
                nc.vector.tensor_scalar(
                    out=y1sg[sgi][:, gl, :], in0=tmp2[:],
                    scalar1=0.0, scalar2=dinv[:, g : g + 1],
                    op0=ALU.max, op1=ALU.mult,
                )
                if gl == ng - 1:
                    gs = g - gl
                    nc.sync.dma_start(
                        out=agin.rearrange("(n p) f -> p n f", p=P)[:, gs : gs + ng, :],
                        in_=y1sg[sgi][:],
                    )

            if stage >= 3:
                agg_layer(table1, 1, epi1, nsg=1 if stage == 3 else len(sgs))
            if stage >= 5:
                nc.gpsimd.collective_compute(
                    "AllGather", ALU.bypass, ins=[agin[:]], outs=[table2[:]],
                    replica_groups=[list(range(NCORES))],
                )

            # ---- L2 ----
            outsg = {}

            def epi2(sgi, g, gl, ng, region):
                if gl == 0:
                    outsg[sgi] = sb.tile([P, ng, OUT], F32, name="outs", tag="outs", bufs=2)
                a2 = sb.tile([HID, P], BF16, tag="a2", bufs=2, name="a2")
                nc.vector.tensor_copy(out=a2[:], in_=region)
                psf = psX.tile([P, OUT], F32, tag="px", name="psf", bufs=2)
                nc.tensor.matmul(out=psf[:], lhsT=a2[:], rhs=W2b[:], start=True, stop=True)
                tmp = sb.tile([P, OUT], F32, tag="epi2a", bufs=2, name="tmp3")
                nc.scalar.activation(tmp[:], psf[:], AF.Copy, scale=dinv[:, g : g + 1])
                nc.vector.tensor_tensor(
                    out=outsg[sgi][:, gl, :], in0=tmp[:], in1=b2t[:], op=ALU.add
                )
                if gl == ng - 1:
                    gs = g - gl
                    nc.sync.dma_start(out=y[:, gs : gs + ng, :], in_=outsg[sgi][:])

            if stage >= 6:
                agg_layer(table2, 2, epi2, nsg=1 if stage == 6 else len(sgs))

    nc.finalize()
    return nc


def make_in_maps(inputs, sched, idx_list, colv_list, dego_list):
    x = np.asarray(inputs["x"], np.float32)
    W1 = np.asarray(inputs["W1"], np.float32)
    W2 = np.asarray(inputs["W2"], np.float32)
    b1 = np.asarray(inputs["b1"], np.float32)
    b2 = np.asarray(inputs["b2"], np.float32)
    iota_np = np.tile(np.arange(P, dtype=ml_dtypes.bfloat16)[None, :], (P, 1))
    b1r = np.tile(b1[None, :], (P, 1)).astype(np.float32)
    b2r = np.tile(b2[None, :], (P, 1)).astype(np.float32)
    in_maps = []
    for c in range(NCORES):
        xs = np.zeros((P, PPC), np.float32)
        xs[:, :TPC] = x[c * TPC : (c + 1) * TPC].T
        in_maps.append(
            {
                "xT": xs,
                "dego": dego_list[c],
                "W1": W1,
                "W2": W2,
                "b1r": b1r,
                "b2r": b2r,
                "iota": iota_np,
                "idx": idx_list[c],
                "colv": colv_list[c],
            }
        )
    return in_maps


def assemble_output(results):
    outs = []
    for c in range(NCORES):
        yc = results[c]["y"]
        yc = np.transpose(yc, (1, 0, 2)).reshape(PPC, OUT)[:TPC]
        outs.append(yc)
    return np.concatenate(outs, axis=0)


def kernel(**inputs):
    sched, idx_list, colv_list, dego_list, _ = host_prep(inputs["edge_index"])
    nc = build_kernel(sched)
    in_maps = make_in_maps(inputs, sched, idx_list, colv_list, dego_list)
    res = run_bass_kernel_spmd(nc, in_maps, core_ids=list(range(NCORES)))
    return assemble_output(res.results)



# revision 18
# speedup vs baseline: 5.2670x; 5.2670x over previous
"""GCN forward on 8 TRN2 NeuronCores — scatter-add based, instruction-count minimized.

Model (reference.py): 2-layer GCN, N=100000 nodes, E=1600000 edges,
IN=HID=128, OUT=64, symmetric normalization with self-loops.

Math:
  deg[t] = in_degree(t)+1 ; dinv = deg^-1/2
  table1[s] = dinv[s]*(x@W1)[s]
  y1[t]     = relu(dinv[t]*sum_e table1[row_e] + b1)
  table2[s] = dinv[s]*y1[s]
  out[t]    = (dinv[t]*sum_e table2[row_e])@W2 + b2

Key runtime fact (measured): this runtime has ~40us PER-INSTRUCTION dispatch
overhead, so the design minimizes instruction count:
  - segment-sum via dma_scatter_add (f32, CCE add) instead of one-hot matmuls
  - edges packed into rank-pair calls so no call has duplicate target rows
    (HW CCE read-modify-write would race on dups); pads go to trash rows
  - transposes via a single dma_gather(transpose=True) over the full table
  - epilogues batched 14 blocks per vector op with broadcast APs
"""

import sys

sys.path.insert(0, "/opt/trn_rl_repo")
import numpy as np
import ml_dtypes

import concourse.bass as bass
import concourse.mybir as mybir
import concourse.tile as tile
from concourse import bacc
from concourse.bass_utils import run_bass_kernel_spmd

F32 = mybir.dt.float32
BF16 = mybir.dt.bfloat16
I16 = mybir.dt.int16
AF = mybir.ActivationFunctionType
ALU = mybir.AluOpType

P = 128
N, E = 100000, 1600000
IN, HID, OUT = 128, 128, 64
NCORES = 8
TPC = 12500            # real nodes per core
PPC = 12544            # padded rows per core (98 blocks)
NG = PPC // P          # 98
NPAD = PPC * NCORES    # 100352 table rows
NCHUNK = 4
CHUNK = NPAD // NCHUNK  # 25088 (int16-addressable)
TRASH = 2 * PPC         # acc rows [TRASH, TRASH+640) are scratch for pad slots
ACCROWS = TRASH + 640   # 25728 < 32768


def _wrap16(a):
    """int16 slot array -> [16, n/16] wrapped, replicated to 128 partitions."""
    a = np.asarray(a, np.int16)
    assert a.size % 16 == 0
    return np.tile(a.reshape(-1, 16).T, (8, 1))


def host_prep(edge_index):
    row = np.asarray(edge_index[0], dtype=np.int64)
    col = np.asarray(edge_index[1], dtype=np.int64)
    loop = np.arange(N, dtype=np.int64)
    rows_all = np.concatenate([row, loop])
    cols_all = np.concatenate([col, loop])
    deg = np.bincount(cols_all, minlength=N)  # includes self-loop
    dinv = 1.0 / np.sqrt(deg.astype(np.float64))
    srow_all = (rows_all // TPC) * PPC + (rows_all % TPC)

    owner_all = cols_all // TPC
    l_all = cols_all - owner_all * TPC

    # per-core edge lists with rank within target
    per_core = []
    for c in range(NCORES):
        sel = owner_all == c
        s = srow_all[sel]
        l = l_all[sel]
        order = np.argsort(l, kind="stable")
        s, l = s[order], l[order]
        cnt = np.bincount(l, minlength=TPC)
        starts = np.zeros(TPC, np.int64)
        starts[1:] = np.cumsum(cnt)[:-1]
        rank = np.arange(l.shape[0]) - starts[l]
        per_core.append((s, l, rank))

    maxdeg = max(pc[2].max() + 1 for pc in per_core)
    K = int(-(-maxdeg // 2))  # rank-pair calls

    # per (core, k, ch) counts -> uniform padded sizes
    counts = np.zeros((NCORES, K, NCHUNK), np.int64)
    for c in range(NCORES):
        s, l, rank = per_core[c]
        k = rank >> 1
        ch = s // CHUNK
        np.add.at(counts[c], (k, ch), 1)
    nkc = counts.max(axis=0)
    nkc_pad = ((nkc + P - 1) // P) * P
    nkc_pad = np.maximum(nkc_pad, P)  # keep every sub-gather non-empty
    Gk = nkc_pad.sum(axis=1)  # slots per call
    G_MAX = int(Gk.max())
    call_off = np.zeros(K + 1, np.int64)
    call_off[1:] = np.cumsum(2 * Gk // 16)  # idxpack columns per call
    IDXCOLS = int(call_off[-1])

    # per-core packed idx arrays
    idxpacks = []
    for c in range(NCORES):
        s, l, rank = per_core[c]
        k = (rank >> 1).astype(np.int64)
        parity = (rank & 1).astype(np.int64)
        ch = s // CHUNK
        lidx = (s - ch * CHUNK).astype(np.int64)
        dest = parity * PPC + l
        cols = np.zeros((128, IDXCOLS), np.int16)
        # bucket sort edges by (k, ch)
        key = k * NCHUNK + ch
        order = np.argsort(key, kind="stable")
        key_s = key[order]
        lidx_s = lidx[order]
        dest_s = dest[order]
        bstart = np.searchsorted(key_s, np.arange(K * NCHUNK + 1))
        for kk in range(K):
            gidx = np.zeros(int(Gk[kk]), np.int64)
            sidx = np.zeros(int(Gk[kk]), np.int64)
            # default: gather row 0, scatter to unique trash rows
            sidx[:] = TRASH + (np.arange(int(Gk[kk])) % 512)
            off = 0
            for cc in range(NCHUNK):
                b0, b1 = int(bstart[kk * NCHUNK + cc]), int(bstart[kk * NCHUNK + cc + 1])
                n = b1 - b0
                gidx[off : off + n] = lidx_s[b0:b1]
                sidx[off : off + n] = dest_s[b0:b1]
                off += int(nkc_pad[kk, cc])
            a = call_off[kk]
            g16 = _wrap16(gidx)
            s16 = _wrap16(sidx)
            cols[:, a : a + g16.shape[1]] = g16
            cols[:, a + g16.shape[1] : a + g16.shape[1] + s16.shape[1]] = s16
        idxpacks.append(np.ascontiguousarray(cols))

    sched = {
        "K": K,
        "nkc_pad": nkc_pad,
        "Gk": Gk,
        "G_MAX": G_MAX,
        "call_off": call_off,
        "IDXCOLS": IDXCOLS,
    }
    return sched, idxpacks, dinv


def build_kernel(sched, stage=9):
    K = sched["K"]
    nkc_pad = sched["nkc_pad"]
    Gk = sched["Gk"]
    G_MAX = sched["G_MAX"]
    call_off = sched["call_off"]
    IDXCOLS = sched["IDXCOLS"]
    GB = G_MAX // P  # gather buffer blocks

    nc = bacc.Bacc("TRN2", target_bir_lowering=False, num_devices=NCORES)
    xbf = nc.dram_tensor("xbf", [P, PPC], BF16, kind="ExternalInput")
    W1 = nc.dram_tensor("W1", [IN, HID], F32, kind="ExternalInput")
    W2 = nc.dram_tensor("W2", [HID, OUT], F32, kind="ExternalInput")
    b1r = nc.dram_tensor("b1r", [P, HID], F32, kind="ExternalInput")
    b2c = nc.dram_tensor("b2c", [OUT, 1], F32, kind="ExternalInput")
    dinv_own = nc.dram_tensor("dinv_own", [P, NG], F32, kind="ExternalInput")
    dinv_bT = nc.dram_tensor("dinv_bT", [P, PPC], BF16, kind="ExternalInput")
    idxpack = nc.dram_tensor("idxpack", [P, IDXCOLS], I16, kind="ExternalInput")
    iota128 = nc.dram_tensor("iota128", [P, 8], I16, kind="ExternalInput")
    y = nc.dram_tensor("y", [OUT, PPC], F32, kind="ExternalOutput")

    with tile.TileContext(nc) as tc:
        with (
            tc.tile_pool(name="const", bufs=1) as cpool,
            tc.tile_pool(name="dram", bufs=1, space="DRAM") as dpool,
        ):
            W1b = cpool.tile([IN, HID], BF16)
            nc.gpsimd.dma_start(out=W1b[:], in_=W1[:])
            W2b = cpool.tile([HID, OUT], BF16)
            nc.gpsimd.dma_start(out=W2b[:], in_=W2[:])
            b1t = cpool.tile([P, HID], F32)
            nc.sync.dma_start(out=b1t[:], in_=b1r[:])
            b2t = cpool.tile([OUT, 1], F32)
            nc.sync.dma_start(out=b2t[:], in_=b2c[:])
            dinv_t = cpool.tile([P, NG], F32)
            nc.sync.dma_start(out=dinv_t[:], in_=dinv_own[:])
            iota128_t = cpool.tile([P, 8], I16)
            nc.sync.dma_start(out=iota128_t[:], in_=iota128[:])

            t1in = dpool.tile([PPC, HID], BF16)
            table1 = dpool.tile([NPAD, HID], BF16, addr_space="Shared")
            agin = dpool.tile([PPC, HID], BF16)
            table2 = dpool.tile([NPAD, HID], BF16, addr_space="Shared")
            acc1 = dpool.tile([ACCROWS, HID], F32)
            acc2 = dpool.tile([ACCROWS, HID], F32)
            t1T_d = dpool.tile([P, PPC], BF16)
            y2d = dpool.tile([PPC, HID], BF16)

            # ---- phase A: t1 = dinv*(x@W1), written row-major to t1in ----
            if stage >= 1:
                with (
                    tc.tile_pool(name="pa", bufs=1) as pa,
                    tc.tile_pool(name="paps", bufs=2, space="PSUM") as paps,
                ):
                    xb = pa.tile([P, PPC], BF16)
                    nc.sync.dma_start(out=xb[:], in_=xbf[:])
                    dbt = pa.tile([P, PPC], BF16)
                    nc.sync.dma_start(out=dbt[:], in_=dinv_bT[:])
                    t1T = pa.tile([P, PPC], BF16)
                    for j0 in range(0, PPC, 512):
                        w = min(512, PPC - j0)
                        ps = paps.tile([P, 512], F32, tag="pa", name="ps")
                        nc.tensor.matmul(
                            out=ps[:, :w], lhsT=W1b[:], rhs=xb[:, j0 : j0 + w],
                            start=True, stop=True,
                        )
                        nc.vector.tensor_tensor(
                            out=t1T[:, j0 : j0 + w], in0=ps[:, :w],
                            in1=dbt[:, j0 : j0 + w], op=ALU.mult,
                        )
                    nc.sync.dma_start(out=t1T_d[:], in_=t1T[:])
                    # un-transpose: one gather-transpose over the whole table
                    t1row = pa.tile([P, NG, HID], BF16)
                    nc.gpsimd.dma_gather(
                        t1row[:], t1T_d[:], iota128_t[:], P, P, PPC,
                        transpose=True,
                    )
                    nc.sync.dma_start(
                        out=t1in.rearrange("(n p) f -> p n f", p=P)[:],
                        in_=t1row[:],
                    )
            if stage >= 2:
                nc.gpsimd.collective_compute(
                    "AllGather", ALU.bypass, ins=[t1in[:]], outs=[table1[:]],
                    replica_groups=[list(range(NCORES))],
                )

            # ---- edge aggregation: gather + scatter-add ----
            def agg_layer(table, acc, pool, ncalls):
                fbuf = pool.tile([P, GB, HID], F32, name="fbuf")
                nc.vector.memset(fbuf[:], 0.0)
                nc.sync.dma_start(
                    out=acc.rearrange("(n p) f -> p n f", p=P)[:, : TRASH // P, :],
                    in_=fbuf[:, : TRASH // P, :],
                )
                gbuf = pool.tile([P, GB, HID], BF16, name="gbuf")
                for k in range(ncalls):
                    gk = int(Gk[k])
                    ic = int(2 * gk // 16)
                    idx_t = pool.tile([P, ic], I16, name="idx", tag="idx", bufs=2)
                    nc.sync.dma_start(
                        out=idx_t[:], in_=idxpack[:, call_off[k] : call_off[k] + ic]
                    )
                    off = 0
                    for cc in range(NCHUNK):
                        npad = int(nkc_pad[k, cc])
                        nc.gpsimd.dma_gather(
                            gbuf[:, off // P : (off + npad) // P, :],
                            table[cc * CHUNK : (cc + 1) * CHUNK, :],
                            idx_t[:, off // 16 : (off + npad) // 16],
                            npad, npad, HID,
                            single_packet=False,
                        )
                        off += npad
                    nc.vector.tensor_copy(
                        out=fbuf[:, : gk // P, :], in_=gbuf[:, : gk // P, :]
                    )
                    # the HW SWDGE ring holds 1024 descriptors and a scatter
                    # needs n/8+1 m2s descs; crossing 1024 hangs the exec
                    # unit, and near-capacity mixes with in-flight gathers are
                    # flaky. Keep pieces <=6144 idxs (769 descs).
                    for a in range(0, gk, 6144):
                        b = min(a + 6144, gk)
                        nc.gpsimd.dma_scatter_add(
                            acc[:],
                            fbuf[:, a // P : b // P, :],
                            idx_t[:, (gk + a) // 16 : (gk + b) // 16],
                            b - a, b - a, HID,
                        )

            # ---- L1 ----
            if stage >= 3:
                with tc.tile_pool(name="ag1", bufs=1) as ag1:
                    agg_layer(table1, acc1, ag1, K if stage >= 4 else 1)
            if stage >= 4:
                with tc.tile_pool(name="ep1", bufs=1) as ep1:
                    GRP = 14
                    for g0 in range(0, NG, GRP):
                        ng = min(GRP, NG - g0)
                        h0 = ep1.tile([P, GRP, HID], F32, tag="h0", name="h0", bufs=2)
                        nc.sync.dma_start(
                            out=h0[:, :ng, :],
                            in_=acc1.rearrange("(n p) f -> p n f", p=P)[:, g0 : g0 + ng, :],
                        )
                        h1 = ep1.tile([P, GRP, HID], F32, tag="h1", name="h1", bufs=2)
                        nc.sync.dma_start(
                            out=h1[:, :ng, :],
                            in_=acc1.rearrange("(n p) f -> p n f", p=P)[
                                :, NG + g0 : NG + g0 + ng, :
                            ],
                        )
                        s = ep1.tile([P, GRP, HID], F32, tag="s", name="s", bufs=2)
                        nc.vector.tensor_tensor(
                            out=s[:, :ng, :], in0=h0[:, :ng, :], in1=h1[:, :ng, :],
                            op=ALU.add,
                        )
                        nc.vector.tensor_tensor(
                            out=s[:, :ng, :], in0=s[:, :ng, :],
                            in1=dinv_t[:, g0 : g0 + ng, None].to_broadcast([P, ng, HID]),
                            op=ALU.mult,
                        )
                        nc.vector.tensor_tensor(
                            out=s[:, :ng, :], in0=s[:, :ng, :],
                            in1=b1t[:, None, :].to_broadcast([P, ng, HID]),
                            op=ALU.add,
                        )
                        nc.vector.tensor_scalar(
                            out=s[:, :ng, :], in0=s[:, :ng, :],
                            scalar1=0.0, scalar2=None, op0=ALU.max,
                        )
                        sb = ep1.tile([P, GRP, HID], BF16, tag="sb", name="sb", bufs=2)
                        nc.vector.tensor_tensor(
                            out=sb[:, :ng, :], in0=s[:, :ng, :],
                            in1=dinv_t[:, g0 : g0 + ng, None].to_broadcast([P, ng, HID]),
                            op=ALU.mult,
                        )
                        nc.sync.dma_start(
                            out=agin.rearrange("(n p) f -> p n f", p=P)[:, g0 : g0 + ng, :],
                            in_=sb[:, :ng, :],
                        )
            if stage >= 5:
                nc.gpsimd.collective_compute(
                    "AllGather", ALU.bypass, ins=[agin[:]], outs=[table2[:]],
                    replica_groups=[list(range(NCORES))],
                )

            # ---- L2 ----
            if stage >= 6:
                with tc.tile_pool(name="ag2", bufs=1) as ag2:
                    agg_layer(table2, acc2, ag2, K if stage >= 7 else 1)
            if stage >= 8:
                with (
                    tc.tile_pool(name="ep2", bufs=1) as ep2,
                    tc.tile_pool(name="ep2ps", bufs=2, space="PSUM") as ep2ps,
                ):
                    GRP = 14
                    for g0 in range(0, NG, GRP):
                        ng = min(GRP, NG - g0)
                        h0 = ep2.tile([P, GRP, HID], F32, tag="h0", name="h0b", bufs=2)
                        nc.sync.dma_start(
                            out=h0[:, :ng, :],
                            in_=acc2.rearrange("(n p) f -> p n f", p=P)[:, g0 : g0 + ng, :],
                        )
                        h1 = ep2.tile([P, GRP, HID], F32, tag="h1", name="h1b", bufs=2)
                        nc.sync.dma_start(
                            out=h1[:, :ng, :],
                            in_=acc2.rearrange("(n p) f -> p n f", p=P)[
                                :, NG + g0 : NG + g0 + ng, :
                            ],
                        )
                        s = ep2.tile([P, GRP, HID], F32, tag="s", name="s2", bufs=2)
                        nc.vector.tensor_tensor(
                            out=s[:, :ng, :], in0=h0[:, :ng, :], in1=h1[:, :ng, :],
                            op=ALU.add,
                        )
                        sb = ep2.tile([P, GRP, HID], BF16, tag="sb", name="sb2", bufs=2)
                        nc.vector.tensor_tensor(
                            out=sb[:, :ng, :], in0=s[:, :ng, :],
                            in1=dinv_t[:, g0 : g0 + ng, None].to_broadcast([P, ng, HID]),
                            op=ALU.mult,
                        )
                        nc.sync.dma_start(
                            out=y2d.rearrange("(n p) f -> p n f", p=P)[:, g0 : g0 + ng, :],
                            in_=sb[:, :ng, :],
                        )
                    if stage >= 9:
                        # transpose whole y2d in ONE HWDGE xbar transpose,
                        # then @W2 (transpose-gather crashes for num_idxs>512)
                        y2T = ep2.tile([P, PPC], BF16)
                        nc.sync.dma_start(out=y2T[:], in_=y2d[:], transpose=True)
                        yf = ep2.tile([OUT, PPC], F32)
                        for j0 in range(0, PPC, 512):
                            w = min(512, PPC - j0)
                            ps = ep2ps.tile([OUT, 512], F32, tag="f", name="psf")
                            nc.tensor.matmul(
                                out=ps[:, :w], lhsT=W2b[:],
                                rhs=y2T[:, j0 : j0 + w],
                                start=True, stop=True,
                            )
                            nc.scalar.activation(yf[:, j0 : j0 + w], ps[:, :w], AF.Copy)
                        nc.vector.tensor_tensor(
                            out=yf[:], in0=yf[:],
                            in1=b2t[:, 0:1].to_broadcast([OUT, PPC]), op=ALU.add,
                        )
                        nc.sync.dma_start(out=y[:], in_=yf[:])
                    else:
                        dummy = ep2.tile([OUT, PPC], F32)
                        nc.vector.memset(dummy[:], 0.0)
                        nc.sync.dma_start(out=y[:], in_=dummy[:])

    nc.finalize()
    return nc


def make_in_maps(inputs, sched, idxpacks, dinv):
    x = np.asarray(inputs["x"], np.float32)
    W1 = np.asarray(inputs["W1"], np.float32)
    W2 = np.asarray(inputs["W2"], np.float32)
    b1 = np.asarray(inputs["b1"], np.float32)
    b2 = np.asarray(inputs["b2"], np.float32)
    b1r = np.tile(b1[None, :], (P, 1)).astype(np.float32)
    b2c = np.ascontiguousarray(b2[:, None].astype(np.float32))
    iota128 = _wrap16(np.arange(P, dtype=np.int16))
    in_maps = []
    for c in range(NCORES):
        xs = np.zeros((P, PPC), ml_dtypes.bfloat16)
        xs[:, :TPC] = x[c * TPC : (c + 1) * TPC].T.astype(ml_dtypes.bfloat16)
        dloc = np.ones(PPC, np.float64)
        dloc[:TPC] = dinv[c * TPC : (c + 1) * TPC]
        dinv_own = np.ascontiguousarray(dloc.reshape(NG, P).T.astype(np.float32))
        dinv_bT = np.tile(dloc.astype(ml_dtypes.bfloat16)[None, :], (P, 1))
        in_maps.append(
            {
                "xbf": xs,
                "W1": W1,
                "W2": W2,
                "b1r": b1r,
                "b2c": b2c,
                "dinv_own": dinv_own,
                "dinv_bT": np.ascontiguousarray(dinv_bT),
                "idxpack": idxpacks[c],
                "iota128": iota128,
            }
        )
    return in_maps


def assemble_output(results):
    outs = []
    for c in range(NCORES):
        yc = results[c]["y"]  # [OUT, PPC]
        outs.append(yc[:, :TPC].T)
    return np.ascontiguousarray(np.concatenate(outs, axis=0).astype(np.float32))


def kernel(**inputs):
    sched, idxpacks, dinv = host_prep(inputs["edge_index"])
    nc = build_kernel(sched)
    in_maps = make_in_maps(inputs, sched, idxpacks, dinv)
    res = run_bass_kernel_spmd(nc, in_maps, core_ids=list(range(NCORES)))
    return assemble_output(res.results)
